# revision 22
# baseline (speedup 1.0000x reference)
"""Trainium2 Bass kernel for AdaptivePointMatcher.

Strategy (8 NeuronCores, data-parallel over the N=1024 pred rows, 128 each;
gt_points + MLP params replicated; no collectives):

Host side does pure layout/packing prep (transposes, block-diagonal layout,
bias replication, dtype conversion, fp8 scaling) on the small inputs; all
actual network compute (encoder, pairwise MLP, softmax, matching) runs on
device.

Per core device graph:
  1. Encoder: H^T = relu(W~^T X^T + b1) via a block-diagonal (40,1280) first
     linear (folds all 20 points into one matmul chain), mean-over-P folded
     into W2stack = [w2;w2]/20; then pred_f^T, gt_f^T (feature-major, bf16).
  2. a^T = (pred_f @ W1top)^T * 64 and B^T = (gt_f @ W1bot + b1) * 64.
  3. Main loop over 128 local pred rows i:
       H1 = relu(B^T + a^T[:,i]) -> fp8e4 (DVE 2x mode; some chunks on ACT)
       H2 = W2^T @ H1 (fp8 DoubleRow matmul) -> PSUM, pairs of i share a tile
       H2r = relu(H2) -> bf16 (ACT, one op per i-pair)
       scores = H2r^T @ w3 (4 small matmuls per i, j-major) -> PSUM
       descale -> bf16 scores (j-major), once per 8-row block
  4. Softmax epilogue: PE-transpose scores to i-major, exp (+row sums via
     accum_out), reciprocal, probs, confidence; matched via j-major
     unnormalized-exp matmuls against gt, scaled by 1/sum afterwards.
     exp without max-subtraction is safe: |scores| < 0.01 by construction.
"""

import numpy as np
from contextlib import ExitStack

N, M, P, D = 1024, 512, 20, 128
NCORES = 8
NLOC = N // NCORES  # 128
NCHUNK = (P * 64) // 128  # 10 feature chunks of the folded encoder hidden dim
BLOCK = 8  # pred rows per main-loop block

S1 = 64.0  # scale applied to H1 before fp8
SW2 = 16.0  # scale applied to W2 before fp8
SW3 = 16.0  # scale applied to w3
DESCALE = 1.0 / (S1 * SW2 * SW3)

_CACHE: dict = {}


def _np_dt(dt_name):
    import concourse.mybir as mybir

    return mybir.dt.np(getattr(mybir.dt, dt_name))


def _prep_shared(inputs):
    """Host-side layout/packing of the replicated tensors (all tiny)."""
    bf = _np_dt("bfloat16")
    f8 = _np_dt("float8e4")
    f32 = np.float32
    gt = np.asarray(inputs["gt_points"], f32)  # (512, 20, 2)
    pe_w1 = np.asarray(inputs["pe_w1"], f32)
    pe_b1 = np.asarray(inputs["pe_b1"], f32)
    pe_w2 = np.asarray(inputs["pe_w2"], f32)
    pe_b2 = np.asarray(inputs["pe_b2"], f32)
    mn_w1 = np.asarray(inputs["mn_w1"], f32)
    mn_b1 = np.asarray(inputs["mn_b1"], f32)
    mn_w2 = np.asarray(inputs["mn_w2"], f32)
    mn_w3 = np.asarray(inputs["mn_w3"], f32)

    gt_flat = gt.reshape(M, P * 2)
    out = {}
    out["xt_gt"] = np.ascontiguousarray(gt_flat.T.astype(bf))  # (40, 512)
    out["gtf"] = np.ascontiguousarray(
        gt_flat.reshape(4, 128, P * 2).transpose(1, 0, 2).astype(bf)
    )  # (128, 4, 40): [:, c, :] = gt rows 128c..128c+128
    wt = np.zeros((40, P * 64), f32)
    for p in range(P):
        wt[2 * p : 2 * p + 2, 64 * p : 64 * p + 64] = pe_w1
    out["wt"] = wt.astype(bf)  # (40, 1280)
    out["b1rep"] = np.tile(pe_b1, 2).reshape(128, 1).astype(f32)
    out["b2e"] = pe_b2.reshape(128, 1).astype(f32)
    out["w2s"] = np.concatenate([pe_w2, pe_w2], 0).astype(f32) / P  # (128,128)
    out["w2s"] = out["w2s"].astype(bf)
    out["w1t"] = np.ascontiguousarray(
        (mn_w1[:128] * S1).reshape(128, 2, 128).astype(bf)
    )  # cols chunked: [:, c, :] = mn_w1[:128, 128c:128c+128]*S1
    out["w1b"] = np.ascontiguousarray((mn_w1[128:] * S1).reshape(128, 2, 128).astype(bf))
    out["b1m"] = np.ascontiguousarray((mn_b1 * S1).reshape(2, 128).T.astype(f32))  # (128,2)
    out["w2pk"] = np.ascontiguousarray(
        (mn_w2 * SW2).reshape(2, 128, 128).transpose(1, 0, 2).astype(f8)
    )  # (128,2,128): [:,c,:] = mn_w2[128c:128c+128,:]*SW2
    out["w3pk"] = (mn_w3 * SW3).astype(bf)  # (128,1)
    out["ident"] = np.eye(128, dtype=bf)
    return out


def _prep_pred(inputs, core):
    bf = _np_dt("bfloat16")
    pred = np.asarray(inputs["pred_points"], np.float32).reshape(N, P * 2)
    shard = pred[core * NLOC : (core + 1) * NLOC]  # (128, 40)
    return np.ascontiguousarray(shard.T.astype(bf))  # (40, 128)


def _build_nc():
    import concourse.bass as bass
    import concourse.mybir as mybir
    import concourse.tile as tile
    from concourse import bacc

    fp32 = mybir.dt.float32
    bf16 = mybir.dt.bfloat16
    fp8 = mybir.dt.float8e4
    AF = mybir.ActivationFunctionType
    OP = mybir.AluOpType

    nc = bacc.Bacc(
        "TRN2",
        target_bir_lowering=False,
        debug=False,
        enable_asserts=True,
        num_devices=NCORES,
    )

    # ---- DRAM I/O (host-prepped layouts) ----
    d_xt_pred = nc.dram_tensor("xt_pred", (40, NLOC), bf16, kind="ExternalInput").ap()
    d_xt_gt = nc.dram_tensor("xt_gt", (40, M), bf16, kind="ExternalInput").ap()
    d_gtf = nc.dram_tensor("gtf", (128, 4, P * 2), bf16, kind="ExternalInput").ap()
    d_wt = nc.dram_tensor("wt", (40, P * 64), bf16, kind="ExternalInput").ap()
    d_b1rep = nc.dram_tensor("b1rep", (128, 1), fp32, kind="ExternalInput").ap()
    d_b2e = nc.dram_tensor("b2e", (128, 1), fp32, kind="ExternalInput").ap()
    d_w2s = nc.dram_tensor("w2s", (128, 128), bf16, kind="ExternalInput").ap()
    d_w1t = nc.dram_tensor("w1t", (128, 2, 128), bf16, kind="ExternalInput").ap()
    d_w1b = nc.dram_tensor("w1b", (128, 2, 128), bf16, kind="ExternalInput").ap()
    d_b1m = nc.dram_tensor("b1m", (128, 2), fp32, kind="ExternalInput").ap()
    d_w2pk = nc.dram_tensor("w2pk", (128, 2, 128), fp8, kind="ExternalInput").ap()
    d_w3pk = nc.dram_tensor("w3pk", (128, 1), bf16, kind="ExternalInput").ap()
    d_ident = nc.dram_tensor("ident", (128, 128), bf16, kind="ExternalInput").ap()

    out_matched = nc.dram_tensor("matched", (NLOC, P, 2), fp32, kind="ExternalOutput").ap()
    out_conf = nc.dram_tensor("confidence", (NLOC, 1), fp32, kind="ExternalOutput").ap()
    out_probs = nc.dram_tensor("probs", (NLOC, M), fp32, kind="ExternalOutput").ap()

    with tile.TileContext(nc) as tc, ExitStack() as ctx:
        const = ctx.enter_context(tc.tile_pool(name="const", bufs=1))

        # ---------- persistent tiles + input DMAs ----------
        xt_pred = const.tile([40, NLOC], bf16)
        xt_gt = const.tile([40, M], bf16)
        gtf_bf = const.tile([128, 4, P * 2], bf16)
        wt_bf = const.tile([40, P * 64], bf16)
        b1rep = const.tile([128, 1], fp32)
        b2e = const.tile([128, 1], fp32)
        w2s_bf = const.tile([128, 128], bf16)
        w1t_bf = const.tile([128, 2, 128], bf16)
        w1b_bf = const.tile([128, 2, 128], bf16)
        b1s = const.tile([128, 2], fp32)
        w2pk = const.tile([128, 2, 128], fp8)
        w3pk = const.tile([128, 1], bf16)
        id_bf = const.tile([128, 128], bf16)

        hg_sb = const.tile([128, NCHUNK, M], bf16)
        hp_sb = const.tile([128, NCHUNK, NLOC], bf16)
        predf_sb = const.tile([128, NLOC], bf16)
        gtf_feat = const.tile([128, M], bf16)
        at_sb = const.tile([128, 2, NLOC], fp32)
        bt_sb = const.tile([128, 2, M], fp32)
        scores_sb = const.tile([128, 4, NLOC], bf16)

        nc.sync.dma_start(xt_pred[:], d_xt_pred[:, :])
        nc.sync.dma_start(xt_gt[:], d_xt_gt[:, :])
        nc.sync.dma_start(wt_bf[:], d_wt[:, :])
        nc.sync.dma_start(b1rep[:], d_b1rep[:, :])
        nc.sync.dma_start(b2e[:], d_b2e[:, :])
        nc.sync.dma_start(w2s_bf[:], d_w2s[:, :])
        nc.sync.dma_start(gtf_bf[:], d_gtf[:, :, :])
        nc.sync.dma_start(w1t_bf[:], d_w1t[:, :, :])
        nc.sync.dma_start(w1b_bf[:], d_w1b[:, :, :])
        nc.sync.dma_start(b1s[:], d_b1m[:, :])
        nc.sync.dma_start(w2pk[:], d_w2pk[:, :, :])
        nc.sync.dma_start(w3pk[:], d_w3pk[:, :])
        nc.sync.dma_start(id_bf[:], d_ident[:, :])

        # ---------- encoder ----------
        with tc.tile_pool(name="encpsum", bufs=2, space="PSUM") as encpsum, \
             tc.tile_pool(name="encacc", bufs=1, space="PSUM") as encacc:
            for c in range(NCHUNK):
                lhs = wt_bf[:, 128 * c : 128 * (c + 1)]
                hps = encpsum.tile([128, M], fp32, tag="hps")
                nc.tensor.matmul(hps[:], lhsT=lhs, rhs=xt_gt[:], start=True, stop=True)
                if c % 2 == 0:
                    nc.scalar.activation(hg_sb[:, c, :], hps[:], AF.Relu, bias=b1rep[:])
                else:
                    nc.vector.tensor_scalar(
                        hg_sb[:, c, :], hps[:], b1rep[:, 0:1], 0.0, op0=OP.add, op1=OP.max
                    )
                hpp = encpsum.tile([128, NLOC], fp32, tag="hpp")
                nc.tensor.matmul(hpp[:], lhsT=lhs, rhs=xt_pred[:], start=True, stop=True)
                if c % 2 == 1:
                    nc.scalar.activation(hp_sb[:, c, :], hpp[:], AF.Relu, bias=b1rep[:])
                else:
                    nc.vector.tensor_scalar(
                        hp_sb[:, c, :], hpp[:], b1rep[:, 0:1], 0.0, op0=OP.add, op1=OP.max
                    )

            # layer 2 (+ mean fold)
            pfps = encacc.tile([128, NLOC], fp32, tag="pfps")
            for c in range(NCHUNK):
                nc.tensor.matmul(
                    pfps[:], lhsT=w2s_bf[:], rhs=hp_sb[:, c, :],
                    start=(c == 0), stop=(c == NCHUNK - 1),
                )
            nc.scalar.activation(predf_sb[:], pfps[:], AF.Identity, bias=b2e[:])
            gfps = encacc.tile([128, M], fp32, tag="gfps")
            for c in range(NCHUNK):
                nc.tensor.matmul(
                    gfps[:], lhsT=w2s_bf[:], rhs=hg_sb[:, c, :],
                    start=(c == 0), stop=(c == NCHUNK - 1),
                )
            nc.scalar.activation(gtf_feat[:], gfps[:], AF.Identity, bias=b2e[:])

            # a^T and B^T (scaled x64 via host-scaled w1t/w1b)
            for c in range(2):
                atps = encacc.tile([128, NLOC], fp32, tag="atps")
                nc.tensor.matmul(atps[:], lhsT=w1t_bf[:, c, :], rhs=predf_sb[:], start=True, stop=True)
                nc.vector.tensor_copy(at_sb[:, c, :], atps[:])
                btps = encacc.tile([128, M], fp32, tag="btps")
                nc.tensor.matmul(btps[:], lhsT=w1b_bf[:, c, :], rhs=gtf_feat[:], start=True, stop=True)
                nc.scalar.activation(bt_sb[:, c, :], btps[:], AF.Identity, bias=b1s[:, c : c + 1])

        # ---------- main loop ----------
        h1_pool = ctx.enter_context(tc.tile_pool(name="h1", bufs=BLOCK + 2))
        h2sb_pool = ctx.enter_context(tc.tile_pool(name="h2sb", bufs=4))
        mainps = ExitStack()
        h2ps_pool = mainps.enter_context(tc.tile_pool(name="h2ps", bufs=3, space="PSUM"))
        scps_pool = mainps.enter_context(tc.tile_pool(name="scps", bufs=2, space="PSUM"))
        DR = mybir.MatmulPerfMode.DoubleRow

        # Software-pipelined by one i-pair: pair p's DR matmuls are emitted
        # before pair p-1's L3 matmuls so the PE never waits on ACT's relu.
        NPAIR = NLOC // 2
        PPB = BLOCK // 2  # pairs per scores block
        scps_tiles = {}
        prev = None  # (h2sb, pair_idx) awaiting L3s

        def emit_l3(h2sb, p):
            b = p // PPB
            scps = scps_tiles[b]
            for r in range(2):
                for c in range(4):
                    k = 4 * (2 * (p % PPB) + r) + c
                    nc.tensor.matmul(
                        scps[:, k : k + 1],
                        lhsT=h2sb[:, r, 128 * c : 128 * (c + 1)],
                        rhs=w3pk[:], start=True, stop=True,
                    )

        def emit_copy(b):
            scps = scps_tiles.pop(b)
            nc.vector.tensor_scalar(
                scores_sb[:, :, b * BLOCK : (b + 1) * BLOCK],
                scps[:].rearrange("p (j c) -> p c j", c=4),
                DESCALE, None, op0=OP.mult,
            )

        for p in range(NPAIR):
            if p % PPB == 0:
                scps_tiles[p // PPB] = scps_pool.tile([128, 4 * BLOCK], fp32, tag="scps", name=f"scps{p // PPB}")
            h2ps = h2ps_pool.tile([128, 2, M], fp32)
            h2sb = h2sb_pool.tile([128, 2, M], bf16)
            for r in range(2):
                i = 2 * p + r
                h1 = h1_pool.tile([128, 2, M], fp8)
                if i % 4 == 3:
                    nc.scalar.activation(
                        h1[:, 0, :], bt_sb[:, 0, :], AF.Relu, bias=at_sb[:, 0, i : i + 1]
                    )
                else:
                    nc.vector.tensor_scalar(
                        h1[:, 0, :], bt_sb[:, 0, :], at_sb[:, 0, i : i + 1], 0.0,
                        op0=OP.add, op1=OP.max,
                    )
                nc.vector.tensor_scalar(
                    h1[:, 1, :], bt_sb[:, 1, :], at_sb[:, 1, i : i + 1], 0.0,
                    op0=OP.add, op1=OP.max,
                )
                nc.tensor.matmul(
                    h2ps[:, r, :], lhsT=w2pk[:], rhs=h1[:], perf_mode=DR, start=True, stop=True
                )
            if prev is not None:
                emit_l3(*prev)
                if prev[1] % PPB == PPB - 1:
                    emit_copy(prev[1] // PPB)
            nc.scalar.activation(h2sb[:], h2ps[:], AF.Relu)
            prev = (h2sb, p)
        emit_l3(*prev)
        emit_copy(prev[1] // PPB)

        # ---------- softmax epilogue ----------
        mainps.close()
        epi = ctx.enter_context(tc.tile_pool(name="epi", bufs=1))
        with tc.tile_pool(name="episum", bufs=1, space="PSUM") as episum:
            sct_ps = episum.tile([128, M], bf16, tag="sct")
            for c in range(4):
                nc.tensor.transpose(
                    sct_ps[:, 128 * c : 128 * (c + 1)], scores_sb[:, c, :], id_bf[:]
                )
            exp_sb = epi.tile([128, M], fp32)
            sums = epi.tile([128, 1], fp32)
            nc.scalar.activation(exp_sb[:], sct_ps[:], AF.Exp, accum_out=sums[:])
            rs = epi.tile([128, 1], fp32)
            nc.vector.reciprocal(rs[:], sums[:])
            probs_sb = epi.tile([128, M], fp32)
            nc.vector.tensor_scalar(probs_sb[:], exp_sb[:], rs[:], None, op0=OP.mult)
            nc.sync.dma_start(out_probs[:, :], probs_sb[:])
            conf_sb = epi.tile([128, 1], fp32)
            nc.vector.reduce_max(conf_sb[:], probs_sb[:], axis=mybir.AxisListType.X)
            nc.sync.dma_start(out_conf[:, :], conf_sb[:])

            # matched = (exp @ gt_flat) * rs ; j-major unnormalized exp
            expt_bf = epi.tile([128, 4, NLOC], bf16)
            nc.scalar.activation(expt_bf[:], scores_sb[:], AF.Exp)
            mps = episum.tile([128, P * 2], fp32, tag="mps")
            for c in range(4):
                nc.tensor.matmul(
                    mps[:], lhsT=expt_bf[:, c, :], rhs=gtf_bf[:, c, :],
                    start=(c == 0), stop=(c == 3),
                )
            matched_sb = epi.tile([128, P * 2], fp32)
            nc.vector.tensor_scalar(matched_sb[:], mps[:], rs[:], None, op0=OP.mult)
            nc.sync.dma_start(out_matched.rearrange("n p t -> n (p t)"), matched_sb[:])

    nc.compile()
    return nc


def _get_nc():
    if "nc" not in _CACHE:
        _CACHE["nc"] = _build_nc()
    return _CACHE["nc"]


def make_in_maps(inputs):
    shared = _prep_shared(inputs)
    in_maps = []
    for c in range(NCORES):
        m = dict(shared)
        m["xt_pred"] = _prep_pred(inputs, c)
        in_maps.append(m)
    return in_maps


def kernel(**inputs) -> tuple:
    nc = _get_nc()
    from concourse import bass_utils

    in_maps = make_in_maps(inputs)
    res = bass_utils.run_bass_kernel_spmd(nc, in_maps, core_ids=list(range(NCORES)))
    matched = np.concatenate([r["matched"] for r in res.results], axis=0)
    confidence = np.concatenate([r["confidence"] for r in res.results], axis=0)
    probs = np.concatenate([r["probs"] for r in res.results], axis=0)
    return matched, confidence, probs


# revision 23
# speedup vs baseline: 1.0838x; 1.0838x over previous
"""Trainium2 Bass kernel for AdaptivePointMatcher.

Strategy (8 NeuronCores, data-parallel over the N=1024 pred rows, 128 each;
gt_points + MLP params replicated; no collectives):

Host side does pure layout/packing prep (transposes, block-diagonal layout,
bias replication, dtype conversion, fp8 scaling) on the small inputs; all
actual network compute (encoder, pairwise MLP, softmax, matching) runs on
device.

Per core device graph:
  1. Encoder: H^T = relu(W~^T X^T + b1) via a block-diagonal (40,1280) first
     linear (folds all 20 points into one matmul chain), mean-over-P folded
     into W2stack = [w2;w2]/20; then pred_f^T, gt_f^T (feature-major, bf16).
  2. a^T = (pred_f @ W1top)^T * 64 and B^T = (gt_f @ W1bot + b1) * 64.
  3. Main loop over 128 local pred rows i:
       H1 = relu(B^T + a^T[:,i]) -> fp8e4 (DVE 2x mode; some chunks on ACT)
       H2 = W2^T @ H1 (fp8 DoubleRow matmul) -> PSUM, pairs of i share a tile
       H2r = relu(H2) -> bf16 (ACT, one op per i-pair)
       scores = H2r^T @ w3 (4 small matmuls per i, j-major) -> PSUM
       descale -> bf16 scores (j-major), once per 8-row block
  4. Softmax epilogue: PE-transpose scores to i-major, exp (+row sums via
     accum_out), reciprocal, probs, confidence; matched via j-major
     unnormalized-exp matmuls against gt, scaled by 1/sum afterwards.
     exp without max-subtraction is safe: |scores| < 0.01 by construction.
"""

import numpy as np
from contextlib import ExitStack

N, M, P, D = 1024, 512, 20, 128
NCORES = 8
NLOC = N // NCORES  # 128
NCHUNK = (P * 64) // 128  # 10 feature chunks of the folded encoder hidden dim
BLOCK = 8  # pred rows per main-loop block

S1 = 64.0  # scale applied to H1 before fp8
SW2 = 16.0  # scale applied to W2 before fp8
SW3 = 16.0  # scale applied to w3
DESCALE = 1.0 / (S1 * SW2 * SW3)

_CACHE: dict = {}


def _np_dt(dt_name):
    import concourse.mybir as mybir

    return mybir.dt.np(getattr(mybir.dt, dt_name))


def _prep_shared(inputs):
    """Host-side layout/packing of the replicated tensors (all tiny)."""
    bf = _np_dt("bfloat16")
    f8 = _np_dt("float8e4")
    f32 = np.float32
    gt = np.asarray(inputs["gt_points"], f32)  # (512, 20, 2)
    pe_w1 = np.asarray(inputs["pe_w1"], f32)
    pe_b1 = np.asarray(inputs["pe_b1"], f32)
    pe_w2 = np.asarray(inputs["pe_w2"], f32)
    pe_b2 = np.asarray(inputs["pe_b2"], f32)
    mn_w1 = np.asarray(inputs["mn_w1"], f32)
    mn_b1 = np.asarray(inputs["mn_b1"], f32)
    mn_w2 = np.asarray(inputs["mn_w2"], f32)
    mn_w3 = np.asarray(inputs["mn_w3"], f32)

    gt_flat = gt.reshape(M, P * 2)
    out = {}
    out["xt_gt"] = np.ascontiguousarray(gt_flat.T.astype(bf))  # (40, 512)
    out["gtf"] = np.ascontiguousarray(
        gt_flat.reshape(4, 128, P * 2).transpose(1, 0, 2).astype(bf)
    )  # (128, 4, 40): [:, c, :] = gt rows 128c..128c+128
    wt = np.zeros((40, P * 64), f32)
    for p in range(P):
        wt[2 * p : 2 * p + 2, 64 * p : 64 * p + 64] = pe_w1
    out["wt"] = wt.astype(bf)  # (40, 1280)
    out["b1rep"] = np.tile(pe_b1, 2).reshape(128, 1).astype(f32)
    out["b2e"] = pe_b2.reshape(128, 1).astype(f32)
    out["w2s"] = np.concatenate([pe_w2, pe_w2], 0).astype(f32) / P  # (128,128)
    out["w2s"] = out["w2s"].astype(bf)
    out["w1t"] = np.ascontiguousarray(
        (mn_w1[:128] * S1).reshape(128, 2, 128).astype(bf)
    )  # cols chunked: [:, c, :] = mn_w1[:128, 128c:128c+128]*S1
    out["w1b"] = np.ascontiguousarray((mn_w1[128:] * S1).reshape(128, 2, 128).astype(bf))
    out["b1m"] = np.ascontiguousarray((mn_b1 * S1).reshape(2, 128).T.astype(f32))  # (128,2)
    out["w2pk"] = np.ascontiguousarray(
        (mn_w2 * SW2).reshape(2, 128, 128).transpose(1, 0, 2).astype(f8)
    )  # (128,2,128): [:,c,:] = mn_w2[128c:128c+128,:]*SW2
    out["w3pk"] = (mn_w3 * SW3).astype(bf)  # (128,1)
    out["ident"] = np.eye(128, dtype=bf)
    return out


def _prep_pred(inputs, core):
    bf = _np_dt("bfloat16")
    pred = np.asarray(inputs["pred_points"], np.float32).reshape(N, P * 2)
    shard = pred[core * NLOC : (core + 1) * NLOC]  # (128, 40)
    return np.ascontiguousarray(shard.T.astype(bf))  # (40, 128)


def _build_nc():
    import concourse.bass as bass
    import concourse.mybir as mybir
    import concourse.tile as tile
    from concourse import bacc

    fp32 = mybir.dt.float32
    bf16 = mybir.dt.bfloat16
    fp8 = mybir.dt.float8e4
    AF = mybir.ActivationFunctionType
    OP = mybir.AluOpType

    nc = bacc.Bacc(
        "TRN2",
        target_bir_lowering=False,
        debug=False,
        enable_asserts=True,
        num_devices=NCORES,
    )

    # ---- DRAM I/O (host-prepped layouts) ----
    d_xt_pred = nc.dram_tensor("xt_pred", (40, NLOC), bf16, kind="ExternalInput").ap()
    d_xt_gt = nc.dram_tensor("xt_gt", (40, M), bf16, kind="ExternalInput").ap()
    d_gtf = nc.dram_tensor("gtf", (128, 4, P * 2), bf16, kind="ExternalInput").ap()
    d_wt = nc.dram_tensor("wt", (40, P * 64), bf16, kind="ExternalInput").ap()
    d_b1rep = nc.dram_tensor("b1rep", (128, 1), fp32, kind="ExternalInput").ap()
    d_b2e = nc.dram_tensor("b2e", (128, 1), fp32, kind="ExternalInput").ap()
    d_w2s = nc.dram_tensor("w2s", (128, 128), bf16, kind="ExternalInput").ap()
    d_w1t = nc.dram_tensor("w1t", (128, 2, 128), bf16, kind="ExternalInput").ap()
    d_w1b = nc.dram_tensor("w1b", (128, 2, 128), bf16, kind="ExternalInput").ap()
    d_b1m = nc.dram_tensor("b1m", (128, 2), fp32, kind="ExternalInput").ap()
    d_w2pk = nc.dram_tensor("w2pk", (128, 2, 128), fp8, kind="ExternalInput").ap()
    d_w3pk = nc.dram_tensor("w3pk", (128, 1), bf16, kind="ExternalInput").ap()
    d_ident = nc.dram_tensor("ident", (128, 128), bf16, kind="ExternalInput").ap()

    out_matched = nc.dram_tensor("matched", (NLOC, P, 2), fp32, kind="ExternalOutput").ap()
    out_conf = nc.dram_tensor("confidence", (NLOC, 1), fp32, kind="ExternalOutput").ap()
    out_probs = nc.dram_tensor("probs", (NLOC, M), fp32, kind="ExternalOutput").ap()

    with tile.TileContext(nc) as tc, ExitStack() as ctx:
        const = ctx.enter_context(tc.tile_pool(name="const", bufs=1))

        # ---------- persistent tiles + input DMAs ----------
        xt_pred = const.tile([40, NLOC], bf16)
        xt_gt = const.tile([40, M], bf16)
        gtf_bf = const.tile([128, 4, P * 2], bf16)
        wt_bf = const.tile([40, P * 64], bf16)
        b1rep = const.tile([128, 1], fp32)
        b2e = const.tile([128, 1], fp32)
        w2s_bf = const.tile([128, 128], bf16)
        w1t_bf = const.tile([128, 2, 128], bf16)
        w1b_bf = const.tile([128, 2, 128], bf16)
        b1s = const.tile([128, 2], fp32)
        w2pk = const.tile([128, 2, 128], fp8)
        w3pk = const.tile([128, 1], bf16)
        id_bf = const.tile([128, 128], bf16)

        hg_sb = const.tile([128, NCHUNK, M], bf16)
        hp_sb = const.tile([128, NCHUNK, NLOC], bf16)
        predf_sb = const.tile([128, NLOC], bf16)
        gtf_feat = const.tile([128, M], bf16)
        at_sb = const.tile([128, 2, NLOC], fp32)
        bt_sb = const.tile([128, 2, M], fp32)
        scores_sb = const.tile([128, 4, NLOC], bf16)

        nc.sync.dma_start(xt_pred[:], d_xt_pred[:, :])
        nc.sync.dma_start(xt_gt[:], d_xt_gt[:, :])
        nc.sync.dma_start(wt_bf[:], d_wt[:, :])
        nc.sync.dma_start(b1rep[:], d_b1rep[:, :])
        nc.sync.dma_start(b2e[:], d_b2e[:, :])
        nc.sync.dma_start(w2s_bf[:], d_w2s[:, :])
        nc.sync.dma_start(gtf_bf[:], d_gtf[:, :, :])
        nc.sync.dma_start(w1t_bf[:], d_w1t[:, :, :])
        nc.sync.dma_start(w1b_bf[:], d_w1b[:, :, :])
        nc.sync.dma_start(b1s[:], d_b1m[:, :])
        nc.sync.dma_start(w2pk[:], d_w2pk[:, :, :])
        nc.sync.dma_start(w3pk[:], d_w3pk[:, :])
        nc.sync.dma_start(id_bf[:], d_ident[:, :])

        # ---------- encoder ----------
        with tc.tile_pool(name="encpsum", bufs=2, space="PSUM") as encpsum, \
             tc.tile_pool(name="encacc", bufs=1, space="PSUM") as encacc:
            for c in range(NCHUNK):
                lhs = wt_bf[:, 128 * c : 128 * (c + 1)]
                hps = encpsum.tile([128, M], fp32, tag="hps")
                nc.tensor.matmul(hps[:], lhsT=lhs, rhs=xt_gt[:], start=True, stop=True)
                if c % 2 == 0:
                    nc.scalar.activation(hg_sb[:, c, :], hps[:], AF.Relu, bias=b1rep[:])
                else:
                    nc.vector.tensor_scalar(
                        hg_sb[:, c, :], hps[:], b1rep[:, 0:1], 0.0, op0=OP.add, op1=OP.max
                    )
                hpp = encpsum.tile([128, NLOC], fp32, tag="hpp")
                nc.tensor.matmul(hpp[:], lhsT=lhs, rhs=xt_pred[:], start=True, stop=True)
                if c % 2 == 1:
                    nc.scalar.activation(hp_sb[:, c, :], hpp[:], AF.Relu, bias=b1rep[:])
                else:
                    nc.vector.tensor_scalar(
                        hp_sb[:, c, :], hpp[:], b1rep[:, 0:1], 0.0, op0=OP.add, op1=OP.max
                    )

            # layer 2 (+ mean fold)
            pfps = encacc.tile([128, NLOC], fp32, tag="pfps")
            for c in range(NCHUNK):
                nc.tensor.matmul(
                    pfps[:], lhsT=w2s_bf[:], rhs=hp_sb[:, c, :],
                    start=(c == 0), stop=(c == NCHUNK - 1),
                )
            nc.scalar.activation(predf_sb[:], pfps[:], AF.Identity, bias=b2e[:])
            gfps = encacc.tile([128, M], fp32, tag="gfps")
            for c in range(NCHUNK):
                nc.tensor.matmul(
                    gfps[:], lhsT=w2s_bf[:], rhs=hg_sb[:, c, :],
                    start=(c == 0), stop=(c == NCHUNK - 1),
                )
            nc.scalar.activation(gtf_feat[:], gfps[:], AF.Identity, bias=b2e[:])

            # a^T and B^T (scaled x64 via host-scaled w1t/w1b)
            for c in range(2):
                atps = encacc.tile([128, NLOC], fp32, tag="atps")
                nc.tensor.matmul(atps[:], lhsT=w1t_bf[:, c, :], rhs=predf_sb[:], start=True, stop=True)
                nc.vector.tensor_copy(at_sb[:, c, :], atps[:])
                btps = encacc.tile([128, M], fp32, tag="btps")
                nc.tensor.matmul(btps[:], lhsT=w1b_bf[:, c, :], rhs=gtf_feat[:], start=True, stop=True)
                nc.scalar.activation(bt_sb[:, c, :], btps[:], AF.Identity, bias=b1s[:, c : c + 1])

        # ---------- main loop ----------
        h1_pool = ctx.enter_context(tc.tile_pool(name="h1", bufs=BLOCK + 2))
        h2sb_pool = ctx.enter_context(tc.tile_pool(name="h2sb", bufs=4))
        mainps = ExitStack()
        h2ps_pool = mainps.enter_context(tc.tile_pool(name="h2ps", bufs=3, space="PSUM"))
        scps_pool = mainps.enter_context(tc.tile_pool(name="scps", bufs=2, space="PSUM"))
        DR = mybir.MatmulPerfMode.DoubleRow

        pending_copy = None  # deferred scores-copy: (scps_tile, block_idx)
        for ib in range(NLOC // BLOCK):
            scps = scps_pool.tile([128, 4 * BLOCK], fp32)
            h2sbs = []
            for jp in range(BLOCK // 2):
                h2ps = h2ps_pool.tile([128, 2, M], fp32)
                h2sb = h2sb_pool.tile([128, 2, M], bf16)
                for r in range(2):
                    i = ib * BLOCK + 2 * jp + r
                    h1 = h1_pool.tile([128, 2, M], fp8)
                    if i % 8 == 7 or i % 16 == 3:
                        nc.scalar.activation(
                            h1[:, 0, :], bt_sb[:, 0, :], AF.Relu, bias=at_sb[:, 0, i : i + 1]
                        )
                    else:
                        nc.vector.tensor_scalar(
                            h1[:, 0, :], bt_sb[:, 0, :], at_sb[:, 0, i : i + 1], 0.0,
                            op0=OP.add, op1=OP.max,
                        )
                    nc.vector.tensor_scalar(
                        h1[:, 1, :], bt_sb[:, 1, :], at_sb[:, 1, i : i + 1], 0.0,
                        op0=OP.add, op1=OP.max,
                    )
                    nc.tensor.matmul(
                        h2ps[:, r, :], lhsT=w2pk[:], rhs=h1[:], perf_mode=DR, start=True, stop=True
                    )
                nc.scalar.activation(h2sb[:], h2ps[:], AF.Relu)
                h2sbs.append(h2sb)
            if pending_copy is not None:
                pscps, pib = pending_copy
                nc.scalar.activation(
                    scores_sb[:, :, pib * BLOCK : (pib + 1) * BLOCK],
                    pscps[:].rearrange("p (j c) -> p c j", c=4),
                    AF.Copy, scale=DESCALE,
                )
            for jp in range(BLOCK // 2):
                for r in range(2):
                    for c in range(4):
                        k = 4 * (2 * jp + r) + c
                        nc.tensor.matmul(
                            scps[:, k : k + 1],
                            lhsT=h2sbs[jp][:, r, 128 * c : 128 * (c + 1)],
                            rhs=w3pk[:], start=True, stop=True,
                        )
            pending_copy = (scps, ib)
        pscps, pib = pending_copy
        nc.scalar.activation(
            scores_sb[:, :, pib * BLOCK : (pib + 1) * BLOCK],
            pscps[:].rearrange("p (j c) -> p c j", c=4),
            AF.Copy, scale=DESCALE,
        )

        # ---------- softmax epilogue ----------
        mainps.close()
        epi = ctx.enter_context(tc.tile_pool(name="epi", bufs=1))
        with tc.tile_pool(name="episum", bufs=1, space="PSUM") as episum:
            sct_ps = episum.tile([128, M], bf16, tag="sct")
            for c in range(4):
                nc.tensor.transpose(
                    sct_ps[:, 128 * c : 128 * (c + 1)], scores_sb[:, c, :], id_bf[:]
                )
            exp_sb = epi.tile([128, M], fp32)
            sums = epi.tile([128, 1], fp32)
            nc.scalar.activation(exp_sb[:], sct_ps[:], AF.Exp, accum_out=sums[:])
            rs = epi.tile([128, 1], fp32)
            nc.vector.reciprocal(rs[:], sums[:])
            probs_sb = epi.tile([128, M], fp32)
            nc.vector.tensor_scalar(probs_sb[:], exp_sb[:], rs[:], None, op0=OP.mult)
            nc.sync.dma_start(out_probs[:, :], probs_sb[:])
            conf_sb = epi.tile([128, 1], fp32)
            nc.vector.reduce_max(conf_sb[:], probs_sb[:], axis=mybir.AxisListType.X)
            nc.sync.dma_start(out_conf[:, :], conf_sb[:])

            # matched = (exp @ gt_flat) * rs ; j-major unnormalized exp
            expt_bf = epi.tile([128, 4, NLOC], bf16)
            nc.scalar.activation(expt_bf[:], scores_sb[:], AF.Exp)
            mps = episum.tile([128, P * 2], fp32, tag="mps")
            for c in range(4):
                nc.tensor.matmul(
                    mps[:], lhsT=expt_bf[:, c, :], rhs=gtf_bf[:, c, :],
                    start=(c == 0), stop=(c == 3),
                )
            matched_sb = epi.tile([128, P * 2], fp32)
            nc.vector.tensor_scalar(matched_sb[:], mps[:], rs[:], None, op0=OP.mult)
            nc.sync.dma_start(out_matched.rearrange("n p t -> n (p t)"), matched_sb[:])

    nc.compile()
    return nc


def _get_nc():
    if "nc" not in _CACHE:
        _CACHE["nc"] = _build_nc()
    return _CACHE["nc"]


def make_in_maps(inputs):
    shared = _prep_shared(inputs)
    in_maps = []
    for c in range(NCORES):
        m = dict(shared)
        m["xt_pred"] = _prep_pred(inputs, c)
        in_maps.append(m)
    return in_maps


def kernel(**inputs) -> tuple:
    nc = _get_nc()
    from concourse import bass_utils

    in_maps = make_in_maps(inputs)
    res = bass_utils.run_bass_kernel_spmd(nc, in_maps, core_ids=list(range(NCORES)))
    matched = np.concatenate([r["matched"] for r in res.results], axis=0)
    confidence = np.concatenate([r["confidence"] for r in res.results], axis=0)
    probs = np.concatenate([r["probs"] for r in res.results], axis=0)
    return matched, confidence, probs


# revision 24
# speedup vs baseline: 1.2810x; 1.1819x over previous
"""Trainium2 Bass kernel for AdaptivePointMatcher.

Strategy (8 NeuronCores, data-parallel over the N=1024 pred rows, 128 each;
gt_points + MLP params replicated; no collectives):

Host side does pure layout/packing prep (transposes, block-diagonal layout,
bias replication, dtype conversion, fp8 scaling) on the small inputs; all
actual network compute (encoder, pairwise MLP, softmax, matching) runs on
device.

Per core device graph:
  1. Encoder: H^T = relu(W~^T X^T + b1) via a block-diagonal (40,1280) first
     linear (folds all 20 points into one matmul chain), mean-over-P folded
     into W2stack = [w2;w2]/20; then pred_f^T, gt_f^T (feature-major, bf16).
  2. a^T = (pred_f @ W1top)^T * 64 and B^T = (gt_f @ W1bot + b1) * 64.
  3. Main loop over 128 local pred rows i:
       H1 = relu(B^T + a^T[:,i]) -> fp8e4 (DVE 2x mode; some chunks on ACT)
       H2 = W2^T @ H1 (fp8 DoubleRow matmul) -> PSUM, pairs of i share a tile
       H2r = relu(H2) -> bf16 (ACT, one op per i-pair)
       scores = H2r^T @ w3 (4 small matmuls per i, j-major) -> PSUM
       descale -> bf16 scores (j-major), once per 8-row block
  4. Softmax epilogue: PE-transpose scores to i-major, exp (+row sums via
     accum_out), reciprocal, probs, confidence; matched via j-major
     unnormalized-exp matmuls against gt, scaled by 1/sum afterwards.
     exp without max-subtraction is safe: |scores| < 0.01 by construction.
"""

import numpy as np
from contextlib import ExitStack

N, M, P, D = 1024, 512, 20, 128
NCORES = 8
NLOC = N // NCORES  # 128
NCHUNK = (P * 64) // 128  # 10 feature chunks of the folded encoder hidden dim
BLOCK = 8  # pred rows per main-loop block

S1 = 64.0  # scale applied to H1 before fp8
SW2 = 16.0  # scale applied to W2 before fp8
SW3 = 16.0  # scale applied to w3
DESCALE = 1.0 / (S1 * SW2 * SW3)

_CACHE: dict = {}


def _np_dt(dt_name):
    import concourse.mybir as mybir

    return mybir.dt.np(getattr(mybir.dt, dt_name))


def _prep_shared(inputs):
    """Host-side layout/packing of the replicated tensors (all tiny)."""
    bf = _np_dt("bfloat16")
    f8 = _np_dt("float8e4")
    f32 = np.float32
    gt = np.asarray(inputs["gt_points"], f32)  # (512, 20, 2)
    pe_w1 = np.asarray(inputs["pe_w1"], f32)
    pe_b1 = np.asarray(inputs["pe_b1"], f32)
    pe_w2 = np.asarray(inputs["pe_w2"], f32)
    pe_b2 = np.asarray(inputs["pe_b2"], f32)
    mn_w1 = np.asarray(inputs["mn_w1"], f32)
    mn_b1 = np.asarray(inputs["mn_b1"], f32)
    mn_w2 = np.asarray(inputs["mn_w2"], f32)
    mn_w3 = np.asarray(inputs["mn_w3"], f32)

    gt_flat = gt.reshape(M, P * 2)
    out = {}
    out["xt_gt"] = np.ascontiguousarray(gt_flat.T.astype(bf))  # (40, 512)
    out["gtf"] = np.ascontiguousarray(
        gt_flat.reshape(4, 128, P * 2).transpose(1, 0, 2).astype(bf)
    )  # (128, 4, 40): [:, c, :] = gt rows 128c..128c+128
    wt = np.zeros((40, P * 64), f32)
    for p in range(P):
        wt[2 * p : 2 * p + 2, 64 * p : 64 * p + 64] = pe_w1
    out["wt"] = wt.astype(bf)  # (40, 1280)
    out["b1rep"] = np.tile(pe_b1, 2).reshape(128, 1).astype(f32)
    out["b2e"] = pe_b2.reshape(128, 1).astype(f32)
    out["w2s"] = np.concatenate([pe_w2, pe_w2], 0).astype(f32) / P  # (128,128)
    out["w2s"] = out["w2s"].astype(bf)
    out["w1t"] = np.ascontiguousarray(
        (mn_w1[:128] * S1).reshape(128, 2, 128).astype(bf)
    )  # cols chunked: [:, c, :] = mn_w1[:128, 128c:128c+128]*S1
    out["w1b"] = np.ascontiguousarray((mn_w1[128:] * S1).reshape(128, 2, 128).astype(bf))
    out["b1m"] = np.ascontiguousarray((mn_b1 * S1).reshape(2, 128).T.astype(f32))  # (128,2)
    out["w2pk"] = np.ascontiguousarray(
        (mn_w2 * SW2).reshape(2, 128, 128).transpose(1, 0, 2).astype(f8)
    )  # (128,2,128): [:,c,:] = mn_w2[128c:128c+128,:]*SW2
    out["w3pk"] = (mn_w3 * SW3).astype(bf)  # (128,1)
    out["ident"] = np.eye(128, dtype=bf)
    return out


def _prep_pred(inputs, core):
    bf = _np_dt("bfloat16")
    pred = np.asarray(inputs["pred_points"], np.float32).reshape(N, P * 2)
    shard = pred[core * NLOC : (core + 1) * NLOC]  # (128, 40)
    return np.ascontiguousarray(shard.T.astype(bf))  # (40, 128)


def _build_nc():
    import concourse.bass as bass
    import concourse.mybir as mybir
    import concourse.tile as tile
    from concourse import bacc

    fp32 = mybir.dt.float32
    bf16 = mybir.dt.bfloat16
    fp8 = mybir.dt.float8e4
    AF = mybir.ActivationFunctionType
    OP = mybir.AluOpType

    nc = bacc.Bacc(
        "TRN2",
        target_bir_lowering=False,
        debug=False,
        enable_asserts=True,
        num_devices=NCORES,
    )

    # ---- DRAM I/O (host-prepped layouts) ----
    d_xt_pred = nc.dram_tensor("xt_pred", (40, NLOC), bf16, kind="ExternalInput").ap()
    d_xt_gt = nc.dram_tensor("xt_gt", (40, M), bf16, kind="ExternalInput").ap()
    d_gtf = nc.dram_tensor("gtf", (128, 4, P * 2), bf16, kind="ExternalInput").ap()
    d_wt = nc.dram_tensor("wt", (40, P * 64), bf16, kind="ExternalInput").ap()
    d_b1rep = nc.dram_tensor("b1rep", (128, 1), fp32, kind="ExternalInput").ap()
    d_b2e = nc.dram_tensor("b2e", (128, 1), fp32, kind="ExternalInput").ap()
    d_w2s = nc.dram_tensor("w2s", (128, 128), bf16, kind="ExternalInput").ap()
    d_w1t = nc.dram_tensor("w1t", (128, 2, 128), bf16, kind="ExternalInput").ap()
    d_w1b = nc.dram_tensor("w1b", (128, 2, 128), bf16, kind="ExternalInput").ap()
    d_b1m = nc.dram_tensor("b1m", (128, 2), fp32, kind="ExternalInput").ap()
    d_w2pk = nc.dram_tensor("w2pk", (128, 2, 128), fp8, kind="ExternalInput").ap()
    d_w3pk = nc.dram_tensor("w3pk", (128, 1), bf16, kind="ExternalInput").ap()
    d_ident = nc.dram_tensor("ident", (128, 128), bf16, kind="ExternalInput").ap()

    out_matched = nc.dram_tensor("matched", (NLOC, P, 2), fp32, kind="ExternalOutput").ap()
    out_conf = nc.dram_tensor("confidence", (NLOC, 1), fp32, kind="ExternalOutput").ap()
    out_probs = nc.dram_tensor("probs", (NLOC, M), fp32, kind="ExternalOutput").ap()

    with tile.TileContext(nc) as tc, ExitStack() as ctx:
        const = ctx.enter_context(tc.tile_pool(name="const", bufs=1))

        # ---------- persistent tiles + input DMAs ----------
        xt_pred = const.tile([40, NLOC], bf16)
        xt_gt = const.tile([40, M], bf16)
        gtf_bf = const.tile([128, 4, P * 2], bf16)
        wt_bf = const.tile([40, P * 64], bf16)
        b1rep = const.tile([128, 1], fp32)
        b2e = const.tile([128, 1], fp32)
        w2s_bf = const.tile([128, 128], bf16)
        w1t_bf = const.tile([128, 2, 128], bf16)
        w1b_bf = const.tile([128, 2, 128], bf16)
        b1s = const.tile([128, 2], fp32)
        w2pk = const.tile([128, 2, 128], fp8)
        w3pk = const.tile([128, 1], bf16)
        id_bf = const.tile([128, 128], bf16)

        hg_sb = const.tile([128, NCHUNK, M], bf16)
        hp_sb = const.tile([128, NCHUNK, NLOC], bf16)
        predf_sb = const.tile([128, NLOC], bf16)
        gtf_feat = const.tile([128, M], bf16)
        at_sb = const.tile([128, 2, NLOC], fp32)
        bt_sb = const.tile([128, 2, M], fp32)
        scores_sb = const.tile([128, 4, NLOC], bf16)

        nc.sync.dma_start(xt_pred[:], d_xt_pred[:, :])
        nc.sync.dma_start(xt_gt[:], d_xt_gt[:, :])
        nc.sync.dma_start(wt_bf[:], d_wt[:, :])
        nc.sync.dma_start(b1rep[:], d_b1rep[:, :])
        nc.sync.dma_start(b2e[:], d_b2e[:, :])
        nc.sync.dma_start(w2s_bf[:], d_w2s[:, :])
        nc.sync.dma_start(gtf_bf[:], d_gtf[:, :, :])
        nc.sync.dma_start(w1t_bf[:], d_w1t[:, :, :])
        nc.sync.dma_start(w1b_bf[:], d_w1b[:, :, :])
        nc.sync.dma_start(b1s[:], d_b1m[:, :])
        nc.sync.dma_start(w2pk[:], d_w2pk[:, :, :])
        nc.sync.dma_start(w3pk[:], d_w3pk[:, :])
        nc.sync.dma_start(id_bf[:], d_ident[:, :])

        # ---------- encoder ----------
        with tc.tile_pool(name="encpsum", bufs=2, space="PSUM") as encpsum, \
             tc.tile_pool(name="encacc", bufs=1, space="PSUM") as encacc:
            for c in range(NCHUNK):
                lhs = wt_bf[:, 128 * c : 128 * (c + 1)]
                hps = encpsum.tile([128, M], fp32, tag="hps")
                nc.tensor.matmul(hps[:], lhsT=lhs, rhs=xt_gt[:], start=True, stop=True)
                if c % 2 == 0:
                    nc.scalar.activation(hg_sb[:, c, :], hps[:], AF.Relu, bias=b1rep[:])
                else:
                    nc.vector.tensor_scalar(
                        hg_sb[:, c, :], hps[:], b1rep[:, 0:1], 0.0, op0=OP.add, op1=OP.max
                    )
                hpp = encpsum.tile([128, NLOC], fp32, tag="hpp")
                nc.tensor.matmul(hpp[:], lhsT=lhs, rhs=xt_pred[:], start=True, stop=True)
                if c % 2 == 1:
                    nc.scalar.activation(hp_sb[:, c, :], hpp[:], AF.Relu, bias=b1rep[:])
                else:
                    nc.vector.tensor_scalar(
                        hp_sb[:, c, :], hpp[:], b1rep[:, 0:1], 0.0, op0=OP.add, op1=OP.max
                    )

            # layer 2 (+ mean fold)
            pfps = encacc.tile([128, NLOC], fp32, tag="pfps")
            for c in range(NCHUNK):
                nc.tensor.matmul(
                    pfps[:], lhsT=w2s_bf[:], rhs=hp_sb[:, c, :],
                    start=(c == 0), stop=(c == NCHUNK - 1),
                )
            nc.scalar.activation(predf_sb[:], pfps[:], AF.Identity, bias=b2e[:])
            gfps = encacc.tile([128, M], fp32, tag="gfps")
            for c in range(NCHUNK):
                nc.tensor.matmul(
                    gfps[:], lhsT=w2s_bf[:], rhs=hg_sb[:, c, :],
                    start=(c == 0), stop=(c == NCHUNK - 1),
                )
            nc.scalar.activation(gtf_feat[:], gfps[:], AF.Identity, bias=b2e[:])

            # a^T and B^T (scaled x64 via host-scaled w1t/w1b)
            for c in range(2):
                atps = encacc.tile([128, NLOC], fp32, tag="atps")
                nc.tensor.matmul(atps[:], lhsT=w1t_bf[:, c, :], rhs=predf_sb[:], start=True, stop=True)
                nc.vector.tensor_copy(at_sb[:, c, :], atps[:])
                btps = encacc.tile([128, M], fp32, tag="btps")
                nc.tensor.matmul(btps[:], lhsT=w1b_bf[:, c, :], rhs=gtf_feat[:], start=True, stop=True)
                nc.scalar.activation(bt_sb[:, c, :], btps[:], AF.Identity, bias=b1s[:, c : c + 1])

        # ---------- main loop ----------
        h1_pool = ctx.enter_context(tc.tile_pool(name="h1", bufs=BLOCK + 2))
        h2sb_pool = ctx.enter_context(tc.tile_pool(name="h2sb", bufs=4))
        mainps = ExitStack()
        h2ps_pool = mainps.enter_context(tc.tile_pool(name="h2ps", bufs=3, space="PSUM"))
        scorps_pool = mainps.enter_context(tc.tile_pool(name="scorps", bufs=1, space="PSUM"))
        DR = mybir.MatmulPerfMode.DoubleRow

        # all 512 per-i L3 score columns land in one persistent PSUM bank,
        # laid out (jp, i, jc) so the epilogue can read it j-chunk-major
        scores_ps = scorps_pool.tile([128, NLOC, 4], fp32)
        prev_h2 = None
        for p in range(NLOC // 2):
            h2ps = h2ps_pool.tile([128, 2, M], fp32)
            h2sb = h2sb_pool.tile([128, 2, M], bf16)
            for r in range(2):
                i = 2 * p + r
                h1 = h1_pool.tile([128, 2, M], fp8)
                if i % 8 == 7 or i % 16 == 3:
                    nc.scalar.activation(
                        h1[:, 0, :], bt_sb[:, 0, :], AF.Relu, bias=at_sb[:, 0, i : i + 1]
                    )
                else:
                    nc.vector.tensor_scalar(
                        h1[:, 0, :], bt_sb[:, 0, :], at_sb[:, 0, i : i + 1], 0.0,
                        op0=OP.add, op1=OP.max,
                    )
                nc.vector.tensor_scalar(
                    h1[:, 1, :], bt_sb[:, 1, :], at_sb[:, 1, i : i + 1], 0.0,
                    op0=OP.add, op1=OP.max,
                )
                nc.tensor.matmul(
                    h2ps[:, r, :], lhsT=w2pk[:], rhs=h1[:], perf_mode=DR, start=True, stop=True
                )
            if prev_h2 is not None:
                ph2sb, pp = prev_h2
                for r in range(2):
                    for c in range(4):
                        nc.tensor.matmul(
                            scores_ps[:, 2 * pp + r, c : c + 1],
                            lhsT=ph2sb[:, r, 128 * c : 128 * (c + 1)],
                            rhs=w3pk[:], start=True, stop=True,
                        )
            nc.scalar.activation(h2sb[:], h2ps[:], AF.Relu)
            prev_h2 = (h2sb, p)
        ph2sb, pp = prev_h2
        for r in range(2):
            for c in range(4):
                nc.tensor.matmul(
                    scores_ps[:, 2 * pp + r, c : c + 1],
                    lhsT=ph2sb[:, r, 128 * c : 128 * (c + 1)],
                    rhs=w3pk[:], start=True, stop=True,
                )
        # one descale pass PSUM -> bf16 scores (j-major)
        nc.scalar.activation(
            scores_sb[:],
            scores_ps[:].rearrange("p i c -> p c i"),
            AF.Copy, scale=DESCALE,
        )

        # ---------- softmax epilogue ----------
        mainps.close()
        epi = ctx.enter_context(tc.tile_pool(name="epi", bufs=1))
        with tc.tile_pool(name="episum", bufs=1, space="PSUM") as episum:
            sct_ps = episum.tile([128, M], bf16, tag="sct")
            for c in range(4):
                nc.tensor.transpose(
                    sct_ps[:, 128 * c : 128 * (c + 1)], scores_sb[:, c, :], id_bf[:]
                )
            exp_sb = epi.tile([128, M], fp32)
            sums = epi.tile([128, 1], fp32)
            nc.scalar.activation(exp_sb[:], sct_ps[:], AF.Exp, accum_out=sums[:])
            rs = epi.tile([128, 1], fp32)
            nc.vector.reciprocal(rs[:], sums[:])
            probs_sb = epi.tile([128, M], fp32)
            nc.vector.tensor_scalar(probs_sb[:], exp_sb[:], rs[:], None, op0=OP.mult)
            nc.sync.dma_start(out_probs[:, :], probs_sb[:])
            conf_sb = epi.tile([128, 1], fp32)
            nc.vector.reduce_max(conf_sb[:], probs_sb[:], axis=mybir.AxisListType.X)
            nc.sync.dma_start(out_conf[:, :], conf_sb[:])

            # matched = (exp @ gt_flat) * rs ; j-major unnormalized exp
            expt_bf = epi.tile([128, 4, NLOC], bf16)
            nc.scalar.activation(expt_bf[:], scores_sb[:], AF.Exp)
            mps = episum.tile([128, P * 2], fp32, tag="mps")
            for c in range(4):
                nc.tensor.matmul(
                    mps[:], lhsT=expt_bf[:, c, :], rhs=gtf_bf[:, c, :],
                    start=(c == 0), stop=(c == 3),
                )
            matched_sb = epi.tile([128, P * 2], fp32)
            nc.vector.tensor_scalar(matched_sb[:], mps[:], rs[:], None, op0=OP.mult)
            nc.sync.dma_start(out_matched.rearrange("n p t -> n (p t)"), matched_sb[:])

    nc.compile()
    return nc


def _get_nc():
    if "nc" not in _CACHE:
        _CACHE["nc"] = _build_nc()
    return _CACHE["nc"]


def make_in_maps(inputs):
    shared = _prep_shared(inputs)
    in_maps = []
    for c in range(NCORES):
        m = dict(shared)
        m["xt_pred"] = _prep_pred(inputs, c)
        in_maps.append(m)
    return in_maps


def kernel(**inputs) -> tuple:
    nc = _get_nc()
    from concourse import bass_utils

    in_maps = make_in_maps(inputs)
    res = bass_utils.run_bass_kernel_spmd(nc, in_maps, core_ids=list(range(NCORES)))
    matched = np.concatenate([r["matched"] for r in res.results], axis=0)
    confidence = np.concatenate([r["confidence"] for r in res.results], axis=0)
    probs = np.concatenate([r["probs"] for r in res.results], axis=0)
    return matched, confidence, probs
